# revision 17
# baseline (speedup 1.0000x reference)
"""Trainium2 Bass kernel for nn_Attention_3264175145451.

Full (unsharded) inputs in, full output out. Data-parallel over batch:
16 images / 8 cores = 2 images per core, no collectives.

Per-core pipeline (per image, n=1024 tokens, c=512, H=8 heads, d=64):
  x -> x^T (PE transpose) -> Q^T,K^T (head-pair layout) + V (natural,
  ones-augmented) -> per head-pair: sim^T = K^T q (packed K=64 matmuls),
  E = exp(sim/8) on ScalarE, out'^T = [V|1]^T E accumulated over j-tiles
  (row 64 = softmax denom), normalize with reciprocal + gpsimd partition
  broadcast -> output projection (natural orientation) + bias + residual.
All matmuls run in float32r (TF32-like, 1 cycle/row).
"""

import os
import sys

sys.path.insert(0, "/opt/trn_rl_repo")

import numpy as np

import concourse.bass as bass  # noqa: F401  (engine types)
import concourse.mybir as mybir
import concourse.tile as tile
from concourse import bacc
from concourse.bass_utils import run_bass_kernel_spmd
from concourse.masks import make_identity

F32 = mybir.dt.float32
F32R = mybir.dt.float32r
BF = mybir.dt.bfloat16
AF = mybir.ActivationFunctionType
OP = mybir.AluOpType

B = 16           # total batch
NB = 2           # batches per core
N = 1024         # tokens per image (32*32)
C = 512          # channels
H = 8            # heads
D = 64           # head dim
NCORES = 8

TRACE = bool(int(os.environ.get("BASS_ATTN_TRACE", "0")))

_cache = {}


def _register_ntff_hook():
    """Register the axon NTFF profile hook if the image lacks antenv.axon_hooks."""
    import types

    try:
        from antenv.axon_hooks import get_axon_ntff_profile_hook  # noqa: F401
        return
    except ImportError:
        pass
    try:
        from trn_agent_boot.trn_boot import _ntff_profile_via_ctypes

        hook = _ntff_profile_via_ctypes("/opt/axon/libaxon_pjrt.so")
        mod = types.ModuleType("antenv.axon_hooks")
        mod.get_axon_ntff_profile_hook = lambda: hook
        sys.modules["antenv.axon_hooks"] = mod
    except Exception:
        pass


def build_nc():
    nc = bacc.Bacc("TRN2", target_bir_lowering=False, debug=False,
                   num_devices=NCORES)

    x_ext = nc.dram_tensor("x", [NB, N, C], F32, kind="ExternalInput").ap()
    wqkv_ext = nc.dram_tensor("w_qkv", [C, 3 * C], F32, kind="ExternalInput").ap()
    bqkv_ext = nc.dram_tensor("b_qkv", [3 * C], F32, kind="ExternalInput").ap()
    wout_ext = nc.dram_tensor("w_out", [C, C], F32, kind="ExternalInput").ap()
    bout_ext = nc.dram_tensor("b_out", [C], F32, kind="ExternalInput").ap()
    y_ext = nc.dram_tensor("y", [NB, N, C], F32, kind="ExternalOutput").ap()

    with tile.TileContext(nc) as tc:
        _body(nc, tc, x_ext, wqkv_ext, bqkv_ext, wout_ext, bout_ext, y_ext)
    nc.finalize()
    return nc


def _body(nc, tc, x_ext, wqkv_ext, bqkv_ext, wout_ext, bout_ext, y_ext):
    from contextlib import ExitStack

    ctx = ExitStack()
    with ctx:
        wp = ctx.enter_context(tc.tile_pool(name="wp", bufs=1))
        persist = ctx.enter_context(tc.tile_pool(name="persist", bufs=2))
        xnp = ctx.enter_context(tc.tile_pool(name="xnp", bufs=3))
        ep = ctx.enter_context(tc.tile_pool(name="ep", bufs=8))
        rp = ctx.enter_context(tc.tile_pool(name="rp", bufs=2))
        rbp = ctx.enter_context(tc.tile_pool(name="rbp", bufs=2))
        tbp = ctx.enter_context(tc.tile_pool(name="tbp", bufs=2))
        yp = ctx.enter_context(tc.tile_pool(name="yp", bufs=3))
        drp = ctx.enter_context(tc.tile_pool(name="drp", bufs=4, space="DRAM"))
        psw = ctx.enter_context(tc.tile_pool(name="psw", bufs=2, space="PSUM"))
        pso = ctx.enter_context(tc.tile_pool(name="pso", bufs=2, space="PSUM"))

        # ---- constants / weights (loaded once) ----
        # w_qkv viewed [c, h, t, d]; t: 0=q, 1=k, 2=v.
        # wq/wk lhsT layout [p, kt, (h d)] so Q^T/K^T come out with heads
        # contiguous: partition-pair layout (head 2g -> rows 0-63 of ptile g).
        wq_sb = wp.tile([128, 4, C], BF, tag="wq")
        wk_sb = wp.tile([128, 4, C], BF, tag="wk")
        wv_sb = wp.tile([128, 4, C], BF, tag="wv")
        wqkv_v = wqkv_ext.rearrange("(kt p) (h t d) -> t p kt h d",
                                    kt=4, p=128, h=H, t=3, d=D)
        for w_sb, t in ((wq_sb, 0), (wk_sb, 1), (wv_sb, 2)):
            for kt in range(4):
                nc.gpsimd.dma_start(
                    out=w_sb[:, kt].rearrange("p (h d) -> p h d", h=H),
                    in_=wqkv_v[t][:, kt])

        wo_sb = wp.tile([128, 4, C], BF, tag="wo")
        nc.gpsimd.dma_start(
            out=wo_sb[:],
            in_=wout_ext.rearrange("(kt p) f -> p kt f", p=128))

        # per-partition bias columns for Q^T / K^T m-tiles
        bq_col = wp.tile([128, 4], F32, tag="bqc")
        bk_col = wp.tile([128, 4], F32, tag="bkc")
        bqkv_v = bqkv_ext.rearrange("(mt hp t d) -> t hp d mt",
                                    mt=4, hp=2, t=3, d=D)
        for b_col, t in ((bq_col, 0), (bk_col, 1)):
            for hp in range(2):
                nc.sync.dma_start(out=b_col[bass.ts(hp, 64), :],
                                  in_=bqkv_v[t][hp])

        # b_v broadcast over partitions: [128, (h d)] from dram with 0-stride
        bv_bc = wp.tile([128, C], F32, tag="bvb")
        bv_src = bass.AP(tensor=bqkv_ext.tensor, offset=2 * D,
                         ap=[[0, 128], [3 * D, H], [1, D]])
        nc.sync.dma_start(out=bv_bc[:].rearrange("p (h d) -> p h d", h=H),
                          in_=bv_src)
        # b_out broadcast over partitions
        bo_bc = wp.tile([128, C], F32, tag="bob")
        bo_src = bass.AP(tensor=bout_ext.tensor, offset=0,
                         ap=[[0, 128], [1, C]])
        nc.sync.dma_start(out=bo_bc[:], in_=bo_src)

        for b in range(NB):
            # ---- per-image persistent tiles ----
            xT = persist.tile([128, 4, N], BF, tag="xT")
            q_sb = persist.tile([128, 4, N], BF, tag="q")
            k_sb = persist.tile([128, 4, N], BF, tag="k")
            v_sb = persist.tile([128, 8, H, D + 1], BF, tag="v")
            ot = persist.tile([128, 4, N], BF, tag="ot")

            # ones column of V (softmax denominator trick)
            nc.vector.memset(v_sb[:, :, :, D:D + 1], 1.0)

            # ---- phase A: x -> x^T via bf16 staging + xbar DMA transpose ----
            xbf = drp.tile([N, C], BF, tag="xbf")
            nc.gpsimd.dma_start(out=xbf[:], in_=x_ext[b])
            for ct in range(4):
                nc.sync.dma_start_transpose(xT[:, ct, :],
                                            xbf[:, bass.ts(ct, 128)])

            # ---- phase B: projections ----
            # Q^T, K^T: lhsT = w chunk, rhs = x^T
            for (w_sb, b_col, dst) in ((wq_sb, bq_col, q_sb),
                                       (wk_sb, bk_col, k_sb)):
                for mt in range(4):
                    for ih in range(2):
                        pq = psw.tile([128, 512], F32, tag="work")
                        for kt in range(4):
                            nc.tensor.matmul(
                                pq[:], w_sb[:, kt, bass.ts(mt, 128)],
                                xT[:, kt, bass.ts(ih, 512)],
                                start=(kt == 0), stop=(kt == 3))
                        nc.vector.tensor_scalar_add(
                            dst[:, mt, bass.ts(ih, 512)], pq[:],
                            b_col[:, mt:mt + 1])
            # V natural: lhsT = x^T chunk, rhs = w_v
            for it in range(8):
                pv = psw.tile([128, 512], F32, tag="work")
                for kt in range(4):
                    nc.tensor.matmul(pv[:], xT[:, kt, bass.ts(it, 128)],
                                     wv_sb[:, kt, :],
                                     start=(kt == 0), stop=(kt == 3))
                nc.vector.tensor_tensor(
                    v_sb[:, it, :, 0:D],
                    pv[:].rearrange("p (h d) -> p h d", h=H),
                    bv_bc[:].rearrange("p (h d) -> p h d", h=H), op=OP.add)

            # ---- phase C: attention, one head-pair g at a time ----
            for g in range(4):
                pso_a = pso.tile([D + 1, N], F32, tag="o")
                pso_b = pso.tile([D + 1, N], F32, tag="o")
                for jt in range(8):
                    e_a = ep.tile([128, N], BF, tag="E")
                    e_b = ep.tile([128, N], BF, tag="E")
                    pa = psw.tile([128, 1024], F32, tag="work")
                    pb = psw.tile([128, 1024], F32, tag="work")
                    for ih in range(2):
                        isl = bass.ts(ih, 512)
                        nc.tensor.matmul(pa[:, isl],
                                         k_sb[0:64, g, bass.ts(jt, 128)],
                                         q_sb[0:64, g, isl],
                                         start=True, stop=True)
                        nc.tensor.matmul(pb[:, isl],
                                         k_sb[64:128, g, bass.ts(jt, 128)],
                                         q_sb[64:128, g, isl],
                                         start=True, stop=True)
                    nc.scalar.activation(out=e_a[:], in_=pa[:],
                                         func=AF.Exp, scale=0.125)
                    nc.scalar.activation(out=e_b[:], in_=pb[:],
                                         func=AF.Exp, scale=0.125)
                    for ih in range(2):
                        isl = bass.ts(ih, 512)
                        nc.tensor.matmul(pso_a[:, isl], v_sb[:, jt, 2 * g, :],
                                         e_a[:, isl],
                                         start=(jt == 0), stop=(jt == 7))
                        nc.tensor.matmul(pso_b[:, isl],
                                         v_sb[:, jt, 2 * g + 1, :],
                                         e_b[:, isl],
                                         start=(jt == 0), stop=(jt == 7))
                # normalize: r = 1/s broadcast over the 64 head dims
                rb_a = rbp.tile([64, N], F32, tag="rb")
                rb_b = rbp.tile([64, N], F32, tag="rb")
                for pso_t, rb_t in ((pso_a, rb_a), (pso_b, rb_b)):
                    s_row = rp.tile([128, N], F32, tag="r")
                    nc.vector.tensor_copy(s_row[64:65, :], pso_t[D:D + 1, :])
                    sd = drp.tile([N], F32, tag="sd")
                    nc.sync.dma_start(out=sd[:], in_=s_row[64:65, :])
                    sp = rp.tile([64, 16], F32, tag="sp")
                    nc.sync.dma_start(out=sp[:],
                                      in_=sd[:].rearrange("(p f) -> p f", p=64))
                    rsp = rp.tile([64, 16], F32, tag="rsp")
                    nc.vector.reciprocal(out=rsp[:], in_=sp[:])
                    rd = drp.tile([N], F32, tag="rd")
                    nc.sync.dma_start(out=rd[:].rearrange("(p f) -> p f", p=64),
                                      in_=rsp[:])
                    _rd = rd[:]
                    nc.sync.dma_start(out=rb_t[:], in_=bass.AP(
                        tensor=_rd.tensor, offset=_rd.offset,
                        ap=[[0, 64], [1, N]]))
                # head 2g -> OT partitions 0-63 directly
                nc.vector.tensor_tensor(ot[0:64, g, :], pso_a[0:D, :],
                                        rb_a[:], op=OP.mult)
                # head 2g+1 -> temp, DMA to partitions 64-127
                tb = tbp.tile([64, N], BF, tag="tb")
                nc.vector.tensor_tensor(tb[:], pso_b[0:D, :], rb_b[:],
                                        op=OP.mult)
                nc.sync.dma_start(out=ot[64:128, g, :], in_=tb[:])

            # ---- phase D: output projection + bias + residual ----
            for it in range(8):
                py = psw.tile([128, 512], F32, tag="work")
                for g in range(4):
                    nc.tensor.matmul(py[:], ot[:, g, bass.ts(it, 128)],
                                     wo_sb[:, g, :],
                                     start=(g == 0), stop=(g == 3))
                xi = xnp.tile([128, C], F32, tag="xn")
                nc.sync.dma_start(out=xi[:], in_=x_ext[b, bass.ts(it, 128), :])
                yt = yp.tile([128, C], F32, tag="y")
                nc.vector.tensor_tensor(yt[:], py[:], xi[:], op=OP.add)
                nc.gpsimd.tensor_tensor(yt[:], yt[:], bo_bc[:], op=OP.add)
                nc.sync.dma_start(out=y_ext[b, bass.ts(it, 128), :], in_=yt[:])


def kernel(x, w_qkv, b_qkv, w_out, b_out):
    x = np.ascontiguousarray(np.asarray(x, dtype=np.float32))
    w_qkv = np.ascontiguousarray(np.asarray(w_qkv, dtype=np.float32))
    b_qkv = np.ascontiguousarray(np.asarray(b_qkv, dtype=np.float32))
    w_out = np.ascontiguousarray(np.asarray(w_out, dtype=np.float32))
    b_out = np.ascontiguousarray(np.asarray(b_out, dtype=np.float32))

    bsz, hh, ww, c = x.shape
    assert (bsz, hh, ww, c) == (B, 32, 32, C)
    x_flat = x.reshape(B, N, C)

    if "nc" not in _cache:
        _cache["nc"] = build_nc()
    nc = _cache["nc"]

    if TRACE:
        _register_ntff_hook()

    in_maps = []
    for core in range(NCORES):
        in_maps.append({
            "x": x_flat[NB * core:NB * (core + 1)],
            "w_qkv": w_qkv,
            "b_qkv": b_qkv,
            "w_out": w_out,
            "b_out": b_out,
        })
    res = run_bass_kernel_spmd(nc, in_maps, list(range(NCORES)), trace=TRACE)
    _cache["last_result"] = res
    y = np.concatenate([res.results[i]["y"] for i in range(NCORES)], axis=0)
    return y.reshape(B, 32, 32, C)
